# revision 18
# baseline (speedup 1.0000x reference)
"""Trainium2 Bass kernel for the Bayesian logistic-regression activation matrix.

Computes, for x [N, D], w_mu [D], w_log_var [D], z [NS]:
    mean  = x @ w_mu                       [N]
    var   = (x*x) @ exp(w_log_var)         [N]
    out[i, j] = sqrt(var_i) * z_j + mean_i [N, NS]

Data-parallel over 8 NeuronCores: rows of x sharded, everything else
replicated. Per core (12500 rows): 5 strips of 2500 rows, each strip
5 compute tiles of R=500 rows.

Key design points (all sized against the TRN2 cost model):
  - x is pre-transposed AND pre-cast to bf16 on the host: DRAM tensor
    xt [4 chunks, 128 d, 12500 n]. Strip loads are [128, 2500] slices
    whose DRAM runs are 5000 B contiguous -> full 360 GB/s DMA rate
    (vs 256 B runs at half rate for a device-side transposed load).
    bf16 halves HBM read traffic vs the f32 baseline.
  - squares on DVE, not ACT: sq = x*x via one tensor_tensor per strip.
    All operands bf16+SBUF+packed -> DVE 4x mode (4 elem/cycle/lane),
    ~0.5 us per 2000-col tile. exp(w_log_var) is folded into the var
    matmul's stationary vector instead of pre-scaling x (saves a pass).
  - PE does both d-reductions per tile (K=128 per chunk, PSUM-accum
    over 4 chunks): mean = wb_c^T @ x_c, var = eb_c^T @ sq_c. bf16
    rhs streams at 1 col/cycle -> ~42 us/core total, the PE floor.
  - output is built in [n, j] orientation: out_tile [125 n, 128 j] =
    lhsT(mean/std pairs, stride-4 columns) @ [ones; z]. Four matmuls
    per tile write one PSUM bank [125, 512]; partition p then holds
    output rows 4p..4p+3 CONTIGUOUSLY, so the bf16 store has 1024 B
    DRAM runs -> full-rate 8.9 us/core (vs 17.8 us for a transposed
    [j, n] store at 256 B runs).
  - out is stored bf16 and upcast to f32 on host (halves write traffic;
    total max rel err ~5e-3, well under the 2e-2 gate).
"""

import numpy as np

N = 100000
D = 512
NS = 128
NCORES = 8
NSHARD = N // NCORES  # 12500 rows per core
P = 128               # SBUF partitions
C = D // P            # 4 chunks of the feature dim
S = 500               # rows per strip (DMA granule); strip == compute tile
R = 500               # rows per compute tile; PSUM row [1, 500] f32 fits a bank
G = 4                 # output rows grouped per partition in the store
M = R // G            # 125 out-matmul partitions

_CACHE = {}


def _build_bass(nshard=NSHARD):
    from contextlib import ExitStack

    import concourse.bacc as bacc
    import concourse.mybir as mybir
    import concourse.tile as tile

    f32 = mybir.dt.float32
    bf16 = mybir.dt.bfloat16
    AluOp = mybir.AluOpType

    nstrips = nshard // S
    tps = S // R  # tiles per strip

    nc = bacc.Bacc("TRN2", target_bir_lowering=False, debug=False)

    xt = nc.dram_tensor("xt", [C, P, nshard], bf16, kind="ExternalInput").ap()
    wb = nc.dram_tensor("wb", [P, C], bf16, kind="ExternalInput").ap()
    eb = nc.dram_tensor("eb", [P, C], bf16, kind="ExternalInput").ap()
    zb = nc.dram_tensor("zb", [2, NS], bf16, kind="ExternalInput").ap()
    out = nc.dram_tensor("out", [nshard, NS], bf16, kind="ExternalOutput").ap()

    with tile.TileContext(nc) as tc, ExitStack() as ctx:
        const_pool = ctx.enter_context(tc.tile_pool(name="const", bufs=1))
        xs_pool = ctx.enter_context(tc.tile_pool(name="xs", bufs=4))
        sq_pool = ctx.enter_context(tc.tile_pool(name="sq", bufs=3))
        rows_pool = ctx.enter_context(tc.tile_pool(name="rows", bufs=4))
        osb_pool = ctx.enter_context(tc.tile_pool(name="osb", bufs=3))
        pm_pool = ctx.enter_context(tc.tile_pool(name="pm", bufs=3, space="PSUM"))
        pv_pool = ctx.enter_context(tc.tile_pool(name="pv", bufs=3, space="PSUM"))
        po_pool = ctx.enter_context(tc.tile_pool(name="po", bufs=2, space="PSUM"))

        w_t = const_pool.tile([P, C], bf16)
        nc.sync.dma_start(w_t[:], wb[:])
        e_t = const_pool.tile([P, C], bf16)
        nc.sync.dma_start(e_t[:], eb[:])
        z_t = const_pool.tile([1, NS], bf16)
        nc.sync.dma_start(z_t[:], zb[0:1, :])
        ones_t = const_pool.tile([1, NS], bf16)
        nc.sync.dma_start(ones_t[:], zb[1:2, :])

        for s in range(nstrips):
            s0 = s * S

            # strip load: one 3D DMA, 5000B contiguous DRAM runs
            xs_t = xs_pool.tile([P, C * S], bf16)
            nc.sync.dma_start(
                xs_t[:].rearrange("p (c n) -> p c n", c=C),
                xt[:, :, s0 : s0 + S].rearrange("c p n -> p c n"),
            )

            # whole-strip squares on DVE (4x mode: bf16, SBUF, packed)
            sq_t = sq_pool.tile([P, C * S], bf16)
            nc.vector.tensor_tensor(
                sq_t[:].rearrange("p (c n) -> p c n", c=C),
                xs_t[:].rearrange("p (c n) -> p c n", c=C),
                xs_t[:].rearrange("p (c n) -> p c n", c=C),
                AluOp.mult,
            )

            # one bf16 staging tile for the whole strip's output
            osb_t = osb_pool.tile([M, tps * G * NS], bf16)

            for i in range(tps):
                w0 = i * R

                # d-reductions on PE, PSUM-accumulated over the 4 chunks
                pmean = pm_pool.tile([1, R], f32)
                for c in range(C):
                    nc.tensor.matmul(
                        pmean[:],
                        w_t[:, c : c + 1],
                        xs_t[:, c * S + w0 : c * S + w0 + R],
                        start=(c == 0),
                        stop=(c == C - 1),
                    )
                pvar = pv_pool.tile([1, R], f32)
                for c in range(C):
                    nc.tensor.matmul(
                        pvar[:],
                        e_t[:, c : c + 1],
                        sq_t[:, c * S + w0 : c * S + w0 + R],
                        start=(c == 0),
                        stop=(c == C - 1),
                    )

                # std / mean rows in bf16 (engine SBUF accesses must start at
                # 32-aligned partitions, so they cannot share partitions 0+1
                # of one tile; use two partition-0 tiles + K=1 matmul pairs)
                std_t = rows_pool.tile([1, R], bf16, tag="stdrow")
                nc.scalar.sqrt(std_t[:], pvar[:])
                mean_t = rows_pool.tile([1, R], bf16, tag="meanrow")
                nc.scalar.copy(mean_t[:], pmean[:])

                # out[n, j] = std_n * z_j + mean_n; column b of each
                # 4-row group so partition p holds rows 4p..4p+3
                pout = po_pool.tile([M, G * NS], f32)
                rs = std_t[:].rearrange("p (m b) -> p b m", b=G)
                rm = mean_t[:].rearrange("p (m b) -> p b m", b=G)
                for b in range(G):
                    nc.tensor.matmul(
                        pout[:, b * NS : (b + 1) * NS],
                        rs[:, b],
                        z_t[:],
                        start=True,
                        stop=False,
                    )
                    nc.tensor.matmul(
                        pout[:, b * NS : (b + 1) * NS],
                        rm[:, b],
                        ones_t[:],
                        start=False,
                        stop=True,
                    )

                evict = nc.vector.tensor_copy if (s % 2 == 0) else nc.scalar.copy
                evict(osb_t[:, i * G * NS : (i + 1) * G * NS], pout[:])

            # strip store: one 3D DMA; partition p covers output rows
            # s0 + i*R + 4p .. +3 for each tile i (1KB contiguous runs)
            nc.sync.dma_start(
                out[s0 : s0 + S, :].rearrange("(i p b) j -> p i (b j)", i=tps, b=G),
                osb_t[:],
            )

    nc.compile()
    return nc


def _host_consts(w_mu, w_log_var, z):
    import ml_dtypes

    bf16 = ml_dtypes.bfloat16
    e = np.exp(w_log_var.astype(np.float64))
    wb = np.ascontiguousarray(w_mu.reshape(C, P).T).astype(bf16)
    eb = np.ascontiguousarray(e.reshape(C, P).T).astype(bf16)
    # row 0 pairs with the std row, row 1 with the mean row
    zb = np.stack([z.astype(np.float32), np.ones(NS, dtype=np.float32)]).astype(bf16)
    return wb, eb, zb


def _get_nc():
    if "nc" not in _CACHE:
        _CACHE["nc"] = _build_bass()
    return _CACHE["nc"]


def kernel(x, w_mu, w_log_var, z, _trace=False, _tmpdir=None):
    import ml_dtypes

    from concourse.bass_utils import run_bass_kernel_spmd

    bf16 = ml_dtypes.bfloat16

    x = np.ascontiguousarray(x, dtype=np.float32)
    w_mu = np.asarray(w_mu, dtype=np.float32)
    w_log_var = np.asarray(w_log_var, dtype=np.float32)
    z = np.asarray(z, dtype=np.float32)

    # [8, 512, 12500] chunk-major transposed bf16 copy of x
    xtb = x.reshape(NCORES, NSHARD, D).transpose(0, 2, 1).astype(bf16)

    wb, eb, zb = _host_consts(w_mu, w_log_var, z)

    in_maps = []
    for c in range(NCORES):
        in_maps.append(
            {
                "xt": xtb[c].reshape(C, P, NSHARD),
                "wb": wb,
                "eb": eb,
                "zb": zb,
            }
        )

    nc = _get_nc()
    res = run_bass_kernel_spmd(
        nc,
        in_maps,
        core_ids=list(range(NCORES)),
        trace=_trace,
        tmpdir=_tmpdir,
        stitch_traces=False,
    )
    _CACHE["last_results"] = res
    outs = [np.asarray(r["out"]).astype(np.float32) for r in res.results]
    return np.concatenate(outs, axis=0)
